# revision 31
# baseline (speedup 1.0000x reference)
"""Causal self-attention (B=4, T=2048, C=1024, H=16) on 8 trn2 NeuronCores.

Sharding: 2 heads per core for QKV+attention (tensor-parallel over heads);
two AllToAlls redistribute per-head attention outputs into per-core row
slices for a row-parallel output projection. QKV projection of batch b+1
is interleaved into the ACT-bound attention stream of batch b so the PE
fills its exp-wait gaps. Host does only layout glue (transpose of x,
weight column gather, final concat).
"""

import math
from contextlib import ExitStack

import numpy as np

NCORES = 8
B, T, C = 4, 2048, 1024
H = 16
D = C // H  # 64
HPC = H // NCORES  # heads per core = 2
BT = B * T  # 8192
ROWS_PER_CORE = BT // NCORES  # 1024
HALF_ROWS = ROWS_PER_CORE // 2  # 512 rows per core per A2A half
NKT = T // 128  # 16 k-tiles per batch
NEG = -1.0e30

_compiled = None


def _build(no_collective=False):
    import concourse.tile as tile
    from concourse import bacc, mybir
    from concourse.masks import make_identity, make_lower_triangular

    f32 = mybir.dt.float32
    f32r = mybir.dt.float32r

    nc = bacc.Bacc()

    # ---- DRAM I/O (per-core views; same kernel on all 8 cores) ----
    xt_d = nc.dram_tensor("xt", [C, BT], f32r, kind="ExternalInput")
    wqkv_d = nc.dram_tensor("wqkv", [C, 3 * 128], f32r, kind="ExternalInput")
    bqkv_d = nc.dram_tensor("bqkv", [128, 3], f32, kind="ExternalInput")
    wp_d = nc.dram_tensor("wp", [C, C], f32r, kind="ExternalInput")
    bp_d = nc.dram_tensor("bp", [1, C], f32, kind="ExternalInput")
    ones_d = nc.dram_tensor("ones", [128, NKT * HPC], f32r, kind="ExternalInput")
    out_d = nc.dram_tensor("out", [ROWS_PER_CORE, C], f32, kind="ExternalOutput")

    # internal DRAM for the four collectives (one per batch; shard = 256 rows)
    QROWS = T // NCORES  # 256
    y_loc = [nc.dram_tensor(f"y_loc{q}", [NCORES, 128, QROWS], f32r) for q in range(B)]
    y_all = [nc.dram_tensor(f"y_all{q}", [NCORES, 128, QROWS], f32r) for q in range(B)]

    xt_r = xt_d[:, :].rearrange("(j p) t -> p j t", p=128)  # [128, 8, BT]
    wqkv_r = wqkv_d[:, :].rearrange("(j p) f -> p j f", p=128)  # [128, 8, 384]
    wp_r = wp_d[:, :].rearrange("(j p) f -> p j f", p=128)  # [128, 8, 1024]

    with tile.TileContext(nc) as tc, ExitStack() as ctx:
        qkv_pool = ctx.enter_context(tc.tile_pool(name="qkv_pool", bufs=2))
        wpool = ctx.enter_context(tc.tile_pool(name="wpool", bufs=1))
        xt_pool = ctx.enter_context(tc.tile_pool(name="xt_pool", bufs=3))
        vtmp_pool = ctx.enter_context(tc.tile_pool(name="vtmp", bufs=1))
        pt_pool = ctx.enter_context(tc.tile_pool(name="pt", bufs=3))
        r_pool = ctx.enter_context(tc.tile_pool(name="rp", bufs=1))
        yt_pool = ctx.enter_context(tc.tile_pool(name="yt", bufs=2))
        ytr_pool = ctx.enter_context(tc.tile_pool(name="ytr", bufs=2))
        out_pool = ctx.enter_context(tc.tile_pool(name="op", bufs=2))
        ya_pool = ctx.enter_context(tc.tile_pool(name="ya", bufs=4))
        ps_big = ctx.enter_context(tc.tile_pool(name="ps_big", bufs=3, space="PSUM"))
        ps_yt = ctx.enter_context(tc.tile_pool(name="ps_yt", bufs=1, space="PSUM"))

        # ---- qkv weights + attention constants (needed from the start) ----
        wq_sb = wpool.tile([128, 8, 3 * 128], f32r)
        bias_sb = wpool.tile([128, 3], f32)

        def emit_wq_load():
            for j in range(8):
                nc.sync.dma_start(out=wq_sb[:, j, :], in_=wqkv_r[:, j, :])
            nc.sync.dma_start(out=bias_sb, in_=bqkv_d[:, :])
        mneg = wpool.tile([128, 128], f32, tag="mneg")
        ident = wpool.tile([128, 128], f32, tag="ident")
        # projection weights are declared now but DMA'd later (emit_wp_load)
        wp_sb = wpool.tile([128, 8, C], f32r)
        bp_row = wpool.tile([128, C], f32, tag="bp_row")
        bias_bc = wpool.tile([128, C], f32, tag="bias_bc")

        def emit_wp_load():
            for j in range(8):
                nc.sync.dma_start(out=wp_sb[:, j, :], in_=wp_r[:, j, :])
            nc.sync.dma_start(out=bp_row[0:1, :], in_=bp_d[:, :])
            nc.gpsimd.partition_broadcast(bias_bc[:, :], bp_row[0:1, :])

        def phase1(b):
            """QKV projection for batch b (generator: yields after each
            (tok-tile, m-chunk) psum group; 12 yields). V layout
            [tok128, 65, ktile, slot]; row 64 = ones."""
            qT = qkv_pool.tile([128, T], f32r, tag="qT", name=f"qT{b}")
            kT = qkv_pool.tile([128, T], f32r, tag="kT", name=f"kT{b}")
            V = qkv_pool.tile([128, D + 1, NKT, HPC], f32r, tag="V", name=f"V{b}")
            nc.gpsimd.dma_start(out=V[:, D, :, :], in_=ones_d[:, :])
            result[b] = (qT, kT, V)

            xt_tiles = {}

            def load_xt(tt):
                tok0 = b * T + tt * 512
                xt_t = xt_pool.tile([128, 8, 512], f32r, tag="xt", name=f"xt{b}_{tt}")
                nc.sync.dma_start(out=xt_t[:, 0:4, :], in_=xt_r[:, 0:4, tok0 : tok0 + 512])
                nc.sync.dma_start(out=xt_t[:, 4:8, :], in_=xt_r[:, 4:8, tok0 : tok0 + 512])
                xt_tiles[tt] = xt_t

            load_xt(0)
            if b == 0:
                emit_wq_load()
            for tt in range(4):  # 512-token tiles
                if tt + 1 < 4:
                    load_xt(tt + 1)  # prefetch one tile ahead
                xt_t = xt_tiles.pop(tt)
                for m in range(3):  # q, k, v feature chunks
                    ps = ps_big.tile([128, 1024], f32, tag="big")
                    for j in range(8):
                        nc.tensor.matmul(
                            ps[:, 0:512],
                            wq_sb[:, j, m * 128 : (m + 1) * 128],
                            xt_t[:, j, :],
                            start=(j == 0),
                            stop=(j == 7),
                        )
                    if m == 0:
                        nc.vector.tensor_scalar_add(
                            qT[:, tt * 512 : (tt + 1) * 512], ps[:, 0:512], bias_sb[:, 0:1]
                        )
                    elif m == 1:
                        nc.vector.tensor_scalar_add(
                            kT[:, tt * 512 : (tt + 1) * 512], ps[:, 0:512], bias_sb[:, 1:2]
                        )
                    else:
                        # v chunk: bias-add to SBUF, then PE-transpose back into
                        # the spare second bank of the same psum slot
                        vt_t = vtmp_pool.tile([128, 512], f32)
                        nc.vector.tensor_scalar_add(vt_t[:, :], ps[:, 0:512], bias_sb[:, 2:3])
                        for i in range(4):
                            pv = ps[:, 512 + i * 128 : 640 + i * 128]
                            nc.tensor.transpose(pv, vt_t[:, i * 128 : (i + 1) * 128], ident[:, :])
                            kt_idx = tt * 4 + i
                            for s in range(HPC):
                                nc.vector.tensor_copy(
                                    V[:, 0:D, kt_idx, s], pv[:, s * D : (s + 1) * D]
                                )
                    yield

        def _emit_av(ps_y, V, s, pt, segs, kt, last):
            for lo, hi in segs:
                nc.tensor.matmul(
                    ps_y[0 : D + 1, lo:hi],
                    V[:, :, kt, s],
                    pt[:, lo:hi],
                    start=(kt == 0),
                    stop=last,
                )

        def phase2(b, s):
            """Causal attention for (batch b, head-slot s). Generator:
            yields after each k-tile strip (24 yields)."""
            qT, kT, V = result[b]
            p0 = s * D  # partition base of this head in qT/kT
            for qs in range(2):  # 1024-wide query supertiles
                ps_y = ps_yt.tile([128, 1024], f32, tag="yt", name="ps_y")
                nkt = 8 * (qs + 1)
                prev = None
                for kt in range(nkt):
                    off = max(0, kt * 128 - qs * 1024)
                    if off < 512:
                        segs = [(off, 512), (512, 1024)]
                    else:
                        segs = [(off, 1024)]
                    ps_s = ps_big.tile([128, 1024], f32, tag="big")
                    for lo, hi in segs:
                        nc.tensor.matmul(
                            ps_s[:, lo:hi],
                            kT[p0 : p0 + D, kt * 128 : (kt + 1) * 128],
                            qT[p0 : p0 + D, qs * 1024 + lo : qs * 1024 + hi],
                            start=True,
                            stop=True,
                        )
                    if kt * 128 >= qs * 1024:  # diagonal strip -> causal mask
                        nc.vector.tensor_add(
                            ps_s[:, off : off + 128], ps_s[:, off : off + 128], mneg[:, :]
                        )
                    pt = pt_pool.tile([128, 1024], f32r)
                    nc.scalar.activation(
                        pt[:, off:1024],
                        ps_s[:, off:1024],
                        mybir.ActivationFunctionType.Exp,
                        scale=1.0 / math.sqrt(D),
                    )
                    if prev is not None:
                        _emit_av(ps_y, V, s, *prev, last=False)
                    prev = (pt, segs, kt)
                    yield kt % 3 == 0
                _emit_av(ps_y, V, s, *prev, last=True)

                # free the psum accumulator quickly with one copy, then
                # normalize from SBUF off the slot-critical path
                yt_raw = ytr_pool.tile([128, 1024], f32)
                nc.scalar.copy(yt_raw[0:65, 0:512], ps_y[0:65, 0:512])
                nc.vector.tensor_copy(yt_raw[0:65, 512:1024], ps_y[0:65, 512:1024])
                r_t = r_pool.tile([128, 1024], f32, tag="r")
                nc.vector.reciprocal(r_t[0:1, :], yt_raw[64:65, :])
                rb_t = r_pool.tile([128, 1024], f32, tag="rb")
                nc.gpsimd.partition_broadcast(rb_t[0:64, :], r_t[0:1, :])
                yt_sb = yt_pool.tile([128, 1024], f32r)
                nc.vector.tensor_mul(yt_sb[0:64, :], yt_raw[0:64, :], rb_t[0:64, :])
                for piece in range(4):
                    shard = (qs * 1024 + piece * 256) // 256
                    nc.sync.dma_start(
                        out=y_loc[b][shard, p0 : p0 + D, :],
                        in_=yt_sb[0:64, piece * 256 : (piece + 1) * 256],
                    )

        def emit_a2a(q):
            if no_collective:
                return
            nc.gpsimd.collective_compute(
                "AllToAll",
                mybir.AluOpType.bypass,
                replica_groups=[list(range(NCORES))],
                ins=[y_loc[q][:, :, :]],
                outs=[y_all[q][:, :, :]],
            )

        def proj(q):
            """Output projection for this core's 256 rows of batch q.
            Generator: yields after each of 2 row-tiles."""
            y_src = y_loc[q] if no_collective else y_all[q]
            ya_tiles = {}
            for rt in range(2):
                ps_o = ps_big.tile([128, 1024], f32, tag="big")
                for i in range(8):  # feature chunks (source cores)
                    if rt == 0:
                        ya = ya_pool.tile([128, 256], f32r)
                        nc.sync.dma_start(out=ya, in_=y_src[i, :, :])
                        ya_tiles[i] = ya
                    ya = ya_tiles[i][:, rt * 128 : (rt + 1) * 128]
                    for lo, hi in ((0, 512), (512, 1024)):
                        nc.tensor.matmul(
                            ps_o[:, lo:hi],
                            ya,
                            wp_sb[:, i, lo:hi],
                            start=(i == 0),
                            stop=(i == 7),
                        )
                out_sb = out_pool.tile([128, 1024], f32)
                nc.vector.tensor_add(out_sb[:, :], ps_o[:, :], bias_bc[:, :])
                row = q * 256 + rt * 128
                nc.sync.dma_start(out=out_d[row : row + 128, :], in_=out_sb[:, :])
                yield

        def run_interleaved(primary, filler):
            """Drain `primary`, advancing `filler` at hinted insertion points
            (qsuper pipeline warm-up bubbles and every 4th strip)."""
            for hint in primary:
                if filler is not None and hint:
                    try:
                        next(filler)
                    except StopIteration:
                        filler = None
            return filler

        def drain(gen):
            if gen is not None:
                for _ in gen:
                    pass

        def chain(*gens):
            for g in gens:
                if g is not None:
                    yield from g

        result = {}
        # startup: batch 0 qkv stands alone; constants that phase1 itself
        # doesn't need are emitted after its first group to keep the DMA
        # queues clear at kernel start
        p10 = phase1(0)
        next(p10)
        make_identity(nc, ident[:, :])
        make_lower_triangular(nc, mneg[:, :], val=NEG, diag=False)
        drain(p10)
        for b in range(B):
            # filler work for this batch's attention stream: next batch's
            # qkv projection, then the projection of batch b-2 (whose A2A
            # has long completed)
            parts = []
            if b < B - 1:
                parts.append(phase1(b + 1))
            if b >= 2:
                parts.append(proj(b - 2))
            filler = chain(*parts) if parts else None
            for s in range(HPC):
                filler = run_interleaved(phase2(b, s), filler)
            drain(filler)
            if b == 0:
                emit_wp_load()  # off the critical startup path
            emit_a2a(b)
        drain(proj(2))
        drain(proj(3))

    nc.compile()
    return nc


def _get_compiled():
    global _compiled
    if _compiled is None:
        _compiled = _build()
    return _compiled


def _make_in_maps(x, W_attn, b_attn, W_proj, b_proj):
    xt = np.ascontiguousarray(x.reshape(BT, C).T)  # [C, BT]
    bp = np.ascontiguousarray(b_proj.reshape(1, C))
    ones = np.ones((128, NKT * HPC), dtype=np.float32)
    in_maps = []
    for c in range(NCORES):
        heads = [HPC * c + s for s in range(HPC)]
        cols = []
        for m in range(3):  # q, k, v blocks of W_attn
            for h in heads:
                cols.extend(range(m * C + h * D, m * C + (h + 1) * D))
        cols = np.asarray(cols)
        in_maps.append(
            {
                "xt": xt,
                "wqkv": np.ascontiguousarray(W_attn[:, cols]),
                "bqkv": np.ascontiguousarray(b_attn[cols].reshape(3, 128).T),
                "wp": W_proj,
                "bp": bp,
                "ones": ones,
            }
        )
    return in_maps


def kernel(x, W_attn, b_attn, W_proj, b_proj):
    from concourse.bass_utils import run_bass_kernel_spmd

    x = np.asarray(x, dtype=np.float32)
    W_attn = np.asarray(W_attn, dtype=np.float32)
    b_attn = np.asarray(b_attn, dtype=np.float32)
    W_proj = np.asarray(W_proj, dtype=np.float32)
    b_proj = np.asarray(b_proj, dtype=np.float32)

    nc = _get_compiled()
    in_maps = _make_in_maps(x, W_attn, b_attn, W_proj, b_proj)
    res = run_bass_kernel_spmd(nc, in_maps, core_ids=list(range(NCORES)))

    # core c's output: for each batch q, rows [256c, 256c+256) of that batch
    out = np.empty((BT, C), dtype=np.float32)
    for c in range(NCORES):
        o = res.results[c]["out"]
        for q in range(B):
            out[2048 * q + 256 * c : 2048 * q + 256 * (c + 1)] = o[256 * q : 256 * (q + 1)]
    return out.reshape(B, T, C)
